# revision 20
# baseline (speedup 1.0000x reference)
"""Dense 2-layer 2-head GAT for Trainium2 (Bass/Tile), data-parallel over batch.

Each of the 8 NeuronCores processes one batch element (B=8). The per-head
attention score matrix s[i,j] = leakyrelu(hl_i + hr_j) is rank-1 structured,
so score tiles are generated on-chip (never materialized in DRAM):

  - hl broadcast across partitions comes from a single matmul with a
    column-replicated `a_l` stationary operand against hT.
  - hr enters as a per-partition scalar (DVE tensor_scalar add, or fused as
    the per-partition bias of an ACT Lrelu, or on GPSIMD) - score-tile
    generation is statically spread across DVE/ACT/GPSIMD to balance engines.
  - exp on the scalar engine (ACT), written as float32r so the TensorE
    consumes it at 1 cyc/col.
  - p @ h and the softmax denominator are fp32r matmuls.
  - The diagonal mask is an additive -1e30 eye stripe on the tiles that touch
    the diagonal; softmax runs without max-subtraction (scores bounded ~11,
    exp <= 5e4, fp32-safe; the unnormalized ratio is shift-invariant so
    results match the reference's max-subtracted softmax).

Everything stays in the transposed layout [feat_part, node_free] so each
layer's output feeds the next layer's matmul directly; only the initial x
load and final store transpose via the PE.

Tiles consumed by fp32r matmuls are allocated as float32r and written by ops
directly in that dtype (the BIR verifier requires producers to round);
DVE/ACT consumers of those tiles read them via a bitcast-to-f32 view.
"""

import os
from contextlib import ExitStack

import numpy as np

import concourse.bass as bass
import concourse.mybir as mybir
import concourse.tile as tile
from concourse.alu_op_type import AluOpType
from concourse.masks import make_identity

F32 = mybir.dt.float32
F32R = mybir.dt.float32r
AF = mybir.ActivationFunctionType

N = 2048
F = 256
D = 128
P = 128
ALPHA = 0.2
NEG = -1.0e30
N_CORES = 8


def build_nc(n=N):
    from concourse import bacc
    nc = bacc.Bacc("TRN2", target_bir_lowering=False, debug=False,
                   enable_asserts=False, num_devices=N_CORES)

    x_d = nc.declare_dram_parameter("x", [n, F], F32, isOutput=False)
    W_d, b_d, a_d = {}, {}, {}
    for l in (0, 1):
        for h in (0, 1):
            W_d[l, h] = nc.declare_dram_parameter(f"W_{l}_{h}", [F, D], F32, isOutput=False)
            b_d[l, h] = nc.declare_dram_parameter(f"b_{l}_{h}", [D], F32, isOutput=False)
            a_d[l, h] = nc.declare_dram_parameter(f"a_{l}_{h}", [2 * D, 1], F32, isOutput=False)
    out_d = nc.declare_dram_parameter("out", [n, F], F32, isOutput=True)

    NJ = n // P          # node chunks of 128 (partition dim of score tiles)
    IB = min(512, n)     # i-block width (moving free dim; 512 = one PSUM bank)
    NI = n // IB

    with tile.TileContext(nc) as tc, ExitStack() as ctx:
        const = ctx.enter_context(tc.tile_pool(name="const", bufs=1))
        persist = ctx.enter_context(tc.tile_pool(name="persist", bufs=1))
        headp = ctx.enter_context(tc.tile_pool(name="headp", bufs=2))
        ztp = ctx.enter_context(tc.tile_pool(name="ztp", bufs=6))
        lrp = ctx.enter_context(tc.tile_pool(name="lrp", bufs=6))
        up = ctx.enter_context(tc.tile_pool(name="up", bufs=6))
        epp = ctx.enter_context(tc.tile_pool(name="epp", bufs=2))
        smallp = ctx.enter_context(tc.tile_pool(name="smallp", bufs=4))
        ps_prep = ctx.enter_context(tc.tile_pool(name="ps_prep", bufs=2, space="PSUM"))
        ps_main = ctx.enter_context(tc.tile_pool(name="ps_main", bufs=1, space="PSUM"))
        ps_z = ctx.enter_context(tc.tile_pool(name="ps_z", bufs=1, space="PSUM"))

        # ---- constants ----
        I128 = const.tile([P, P], F32, tag="I128", name="I128")
        make_identity(nc, I128[:])
        dmask = const.tile([P, P], F32, tag="dmask", name="dmask")
        nc.gpsimd.memset(dmask[:], 0.0)
        nc.gpsimd.affine_select(
            out=dmask[:], in_=dmask[:], compare_op=AluOpType.not_equal,
            fill=NEG, base=0, pattern=[[-1, P]], channel_multiplier=1,
        )
        ones_col_f = const.tile([P, 1], F32, tag="ones_col_f", name="ones_col_f")
        nc.vector.memset(ones_col_f[:], 1.0)
        ones_col = const.tile([P, 2], F32R, tag="ones_col", name="ones_col")
        nc.vector.tensor_copy(ones_col[:], ones_col_f[:].to_broadcast([P, 2]))
        ones_row_f = const.tile([1, P], F32, tag="ones_row_f", name="ones_row_f")
        nc.vector.memset(ones_row_f[:], 1.0)
        ones_row = const.tile([1, P], F32R, tag="ones_row", name="ones_row")
        nc.vector.tensor_copy(ones_row[:], ones_row_f[:])

        # ---- parameters (DMA as f32, one rounding copy into f32r) ----
        Wt, bt, Alt, art = {}, {}, {}, {}
        for l in (0, 1):
            for h in (0, 1):
                Wt[l, h] = []
                for c in range(2):
                    wf = smallp.tile([P, D], F32, tag="wload", name="wload")
                    nc.sync.dma_start(out=wf[:], in_=W_d[l, h][c * P:(c + 1) * P, :])
                    w = const.tile([P, D], F32R, tag=f"W{l}{h}{c}", name=f"W{l}{h}{c}")
                    nc.vector.tensor_copy(w[:], wf[:])
                    Wt[l, h].append(w)
                b = const.tile([P, 1], F32, tag=f"b{l}{h}", name=f"b{l}{h}")
                nc.sync.dma_start(
                    out=b[:], in_=b_d[l, h][:].rearrange("(p o) -> p o", o=1))
                bt[l, h] = b
                # a_l replicated across 128 columns so that matmul(lhsT=Al,
                # rhs=hT) emits hl broadcast across partitions. ACT with
                # scale=0 broadcasts the per-partition bias along free dim.
                alf = smallp.tile([P, 1], F32, tag="alload", name="alload")
                nc.sync.dma_start(out=alf[:], in_=a_d[l, h][0:P, 0:1])
                Al = const.tile([P, P], F32R, tag=f"Al{l}{h}", name=f"Al{l}{h}")
                nc.vector.tensor_copy(Al[:], alf[:].to_broadcast([P, P]))
                Alt[l, h] = Al
                arf = smallp.tile([P, 1], F32, tag="arload", name="arload")
                nc.sync.dma_start(out=arf[:], in_=a_d[l, h][P:2 * P, 0:1])
                ar2 = const.tile([P, 2], F32R, tag=f"ar{l}{h}", name=f"ar{l}{h}")
                nc.vector.tensor_copy(ar2[:], arf[:].to_broadcast([P, 2]))
                art[l, h] = ar2

        # ---- load x and transpose to xT [2 x (P, n)] (f32r: feeds hT-mm) ----
        xT = [persist.tile([P, n], F32R, tag=f"xT{f}", name=f"xT{f}") for f in range(2)]
        for c in range(NJ):
            xc = smallp.tile([P, F], F32, tag="xload", name="xload")
            nc.sync.dma_start(out=xc[:], in_=x_d[c * P:(c + 1) * P, :])
            for f in range(2):
                tp = ps_prep.tile([P, IB], F32, tag="prep", name="prep")
                nc.tensor.transpose(tp[:, 0:P], xc[:, f * P:(f + 1) * P], I128[:])
                if (c + f) % 2 == 0:
                    nc.vector.tensor_copy(xT[f][:, c * P:(c + 1) * P], tp[:, 0:P])
                else:
                    nc.scalar.activation(xT[f][:, c * P:(c + 1) * P], tp[:, 0:P], AF.Copy)

        X1T = [persist.tile([P, n], F32R, tag=f"X1T{f}", name=f"X1T{f}") for f in range(2)]
        X2T = [persist.tile([P, n], F32, tag=f"X2T{f}", name=f"X2T{f}") for f in range(2)]

        def gat_head(XT, Wc, b, Al, ar2, OUT, out_f32r):
            # hT[d, i] = sum_f W[f, d] * xT[f, i]  (+ b via ACT Identity bias)
            hT = headp.tile([P, n], F32R, tag="hT", name="hT")
            hTf = hT[:].bitcast(F32)
            for ib in range(NI):
                sl = slice(ib * IB, (ib + 1) * IB)
                ps = ps_prep.tile([P, IB], F32, tag="prep", name="prep")
                nc.tensor.matmul(ps[:], Wc[0][:], XT[0][:, sl], start=True, stop=False)
                nc.tensor.matmul(ps[:], Wc[1][:], XT[1][:, sl], start=False, stop=True)
                nc.scalar.activation(hT[:, sl], ps[:], AF.Identity, bias=b[:])
            # h chunks [node_part, d_free] via PE transpose (f32 path)
            h = headp.tile([P, NJ, P], F32R, tag="h", name="h")
            for jc in range(NJ):
                tp = ps_prep.tile([P, IB], F32, tag="prep", name="prep")
                nc.tensor.transpose(tp[:, 0:P], hTf[:, jc * P:(jc + 1) * P], I128[:])
                if jc % 2 == 0:
                    nc.vector.tensor_copy(h[:, jc, :], tp[:, 0:P])
                else:
                    nc.scalar.activation(h[:, jc, :], tp[:, 0:P], AF.Copy)
            # hl broadcast across partitions: matmul(Al, hT) -> [P, n] (f32)
            hlb = headp.tile([P, n], F32, tag="hlb", name="hlb")
            for ib in range(NI):
                sl = slice(ib * IB, (ib + 1) * IB)
                ps = ps_prep.tile([P, IB], F32, tag="prep", name="prep")
                nc.tensor.matmul(ps[:], Al[:], hT[:, sl], start=True, stop=True)
                nc.vector.tensor_copy(hlb[:, sl], ps[:])
            # hr as columns [P, NJ]: per-chunk N=2 matmuls hT_chunk.T @ [ar ar]
            # (fp32r moving free count must be even, so duplicate the column)
            psr = ps_prep.tile([P, IB], F32, tag="prep", name="prep")
            for jc in range(NJ):
                nc.tensor.matmul(psr[:, 2 * jc:2 * jc + 2],
                                 hT[:, jc * P:(jc + 1) * P],
                                 ar2[:], start=True, stop=True)
            hrc = headp.tile([P, NJ], F32, tag="hrc", name="hrc")
            nc.vector.tensor_copy(
                hrc[:],
                psr[:, 0:2 * NJ].rearrange("p (c t) -> p c t", t=2)[:, :, 0])

            HW_ = min(2 * IB, n)  # macro elementwise width (<= two PSUM banks)
            KH = HW_ // IB
            NH = n // HW_
            for half in range(NH):
                hsl = slice(half * HW_, (half + 1) * HW_)
                oacc = [ps_main.tile([P, IB], F32, tag=f"oacc{k}", name=f"oacc{k}")
                        for k in range(KH)]
                zacc = [ps_z.tile([2, IB], F32, tag=f"zacc{k}", name=f"zacc{k}")
                        for k in range(KH)]
                for jc in range(NJ):
                    zt = ztp.tile([P, HW_], F32, tag="zt", name="zt")
                    # Engine split for score generation: GPSIMD (idle
                    # otherwise, 1-tensor ops only) takes some z-adds; the
                    # leakyrelu runs on DVE except a few chunks on ACT.
                    zeng = nc.gpsimd if jc % 2 == 0 else nc.vector
                    zeng.tensor_scalar_add(zt[:], hlb[:, hsl],
                                           hrc[:, jc:jc + 1])
                    off = jc * P - half * HW_
                    if 0 <= off < HW_:
                        nc.vector.tensor_tensor(zt[:, off:off + P],
                                                zt[:, off:off + P],
                                                dmask[:], AluOpType.add)
                    if jc % 8 == 5:
                        nc.scalar.activation(zt[:], zt[:], AF.Lrelu, alpha=ALPHA)
                    else:
                        nc.vector.scalar_tensor_tensor(
                            zt[:], in0=zt[:], scalar=ALPHA, in1=zt[:],
                            op0=AluOpType.mult, op1=AluOpType.max)
                    u = up.tile([P, HW_], F32R, tag="u", name="u")
                    nc.scalar.activation(u[:], zt[:], AF.Exp)
                    for k in range(KH):
                        nc.tensor.matmul(oacc[k][:], h[:, jc, :],
                                         u[:, k * IB:(k + 1) * IB],
                                         start=(jc == 0), stop=(jc == NJ - 1))
                    for k in range(KH):
                        nc.tensor.matmul(zacc[k][:], ones_col[:],
                                         u[:, k * IB:(k + 1) * IB],
                                         start=(jc == 0), stop=(jc == NJ - 1))
                for k in range(KH):
                    ib = half * KH + k
                    isl = slice(ib * IB, (ib + 1) * IB)
                    recip_f = smallp.tile([1, IB], F32, tag="recip_f", name="recip_f")
                    nc.vector.reciprocal_approx_fast(recip_f[:], zacc[k][0:1, :])
                    recip = smallp.tile([1, IB], F32R, tag="recip", name="recip")
                    nc.vector.tensor_copy(recip[:], recip_f[:])
                    rb = ps_prep.tile([P, IB], F32, tag="prep", name="prep")
                    nc.tensor.matmul(rb[:], ones_row[:], recip[:], start=True, stop=True)
                    rbs = epp.tile([P, IB], F32, tag="rbs", name="rbs")
                    nc.scalar.activation(rbs[:], rb[:], AF.Copy)
                    v = epp.tile([P, IB], F32, tag="v", name="v")
                    nc.vector.tensor_tensor(v[:], oacc[k][:], rbs[:], AluOpType.mult)
                    v2 = epp.tile([P, IB], F32, tag="v2", name="v2")
                    nc.vector.tensor_tensor(v2[:], v[:], hTf[:, isl], AluOpType.add)
                    # elu(v2) = relu(v2) + exp(-relu(-v2)) - 1
                    r1 = epp.tile([P, IB], F32, tag="r1", name="r1")
                    nc.scalar.activation(r1[:], v2[:], AF.Relu, scale=-1.0)
                    r2 = epp.tile([P, IB], F32, tag="r2", name="r2")
                    nc.scalar.activation(r2[:], r1[:], AF.Exp, scale=-1.0)
                    r3 = epp.tile([P, IB], F32, tag="r3", name="r3")
                    nc.scalar.activation(r3[:], v2[:], AF.Relu)
                    nc.vector.scalar_tensor_tensor(
                        OUT[:, isl], in0=r2[:], scalar=-1.0, in1=r3[:],
                        op0=AluOpType.add, op1=AluOpType.add)

        gat_head(xT, Wt[0, 0], bt[0, 0], Alt[0, 0], art[0, 0], X1T[0], True)
        gat_head(xT, Wt[0, 1], bt[0, 1], Alt[0, 1], art[0, 1], X1T[1], True)
        gat_head(X1T, Wt[1, 0], bt[1, 0], Alt[1, 0], art[1, 0], X2T[0], False)
        gat_head(X1T, Wt[1, 1], bt[1, 1], Alt[1, 1], art[1, 1], X2T[1], False)

        # ---- transpose X2T back and store ----
        for c in range(NJ):
            ob = smallp.tile([P, F], F32, tag="ob", name="ob")
            for f in range(2):
                tp = ps_prep.tile([P, IB], F32, tag="prep", name="prep")
                nc.tensor.transpose(tp[:, 0:P], X2T[f][:, c * P:(c + 1) * P], I128[:])
                if (c + f) % 2 == 0:
                    nc.vector.tensor_copy(ob[:, f * P:(f + 1) * P], tp[:, 0:P])
                else:
                    nc.scalar.activation(ob[:, f * P:(f + 1) * P], tp[:, 0:P], AF.Copy)
            nc.sync.dma_start(out=out_d[c * P:(c + 1) * P, :], in_=ob[:])

    nc.compile()
    return nc


_CACHE = {}
LAST_RESULTS = None


def kernel(**inputs):
    global LAST_RESULTS
    from concourse.bass_utils import run_bass_kernel_spmd

    x = np.ascontiguousarray(np.asarray(inputs["x"], dtype=np.float32))
    B = x.shape[0]
    assert B == N_CORES and x.shape[1] == N and x.shape[2] == F

    if "nc" not in _CACHE:
        _CACHE["nc"] = build_nc()
    nc = _CACHE["nc"]

    base = {}
    for l in (0, 1):
        for h in (0, 1):
            base[f"W_{l}_{h}"] = np.ascontiguousarray(
                np.asarray(inputs[f"W_{l}_{h}"], dtype=np.float32))
            base[f"b_{l}_{h}"] = np.ascontiguousarray(
                np.asarray(inputs[f"b_{l}_{h}"], dtype=np.float32))
            base[f"a_{l}_{h}"] = np.ascontiguousarray(
                np.asarray(inputs[f"a_{l}_{h}"], dtype=np.float32))

    in_maps = [dict(base, x=np.ascontiguousarray(x[i])) for i in range(B)]
    res = run_bass_kernel_spmd(nc, in_maps, list(range(N_CORES)),
                               trace=bool(os.environ.get("BASS_TRACE")))
    LAST_RESULTS = res
    out = np.stack([res.results[i]["out"] for i in range(B)], axis=0)
    return out.astype(np.float32)


# revision 25
# speedup vs baseline: 2.4927x; 2.4927x over previous
"""Dense 2-layer 2-head GAT for Trainium2 (Bass/Tile), data-parallel over batch.

Each of the 8 NeuronCores processes one batch element (B=8). The per-head
attention score matrix s[i,j] = leakyrelu(hl_i + hr_j) is rank-1 structured,
so score tiles are generated on-chip (never materialized in DRAM):

  - hl broadcast across partitions comes from a single matmul with a
    column-replicated `a_l` stationary operand against hT.
  - hr enters as a per-partition scalar (DVE tensor_scalar add, or fused as
    the per-partition bias of an ACT Lrelu, or on GPSIMD) - score-tile
    generation is statically spread across DVE/ACT/GPSIMD to balance engines.
  - exp on the scalar engine (ACT), written as float32r so the TensorE
    consumes it at 1 cyc/col.
  - p @ h and the softmax denominator are fp32r matmuls.
  - The diagonal mask is an additive -1e30 eye stripe on the tiles that touch
    the diagonal; softmax runs without max-subtraction (scores bounded ~11,
    exp <= 5e4, fp32-safe; the unnormalized ratio is shift-invariant so
    results match the reference's max-subtracted softmax).

Everything stays in the transposed layout [feat_part, node_free] so each
layer's output feeds the next layer's matmul directly; only the initial x
load and final store transpose via the PE.

Tiles consumed by fp32r matmuls are allocated as float32r and written by ops
directly in that dtype (the BIR verifier requires producers to round);
DVE/ACT consumers of those tiles read them via a bitcast-to-f32 view.
"""

import os
from contextlib import ExitStack

import numpy as np

import concourse.bass as bass
import concourse.mybir as mybir
import concourse.tile as tile
from concourse.alu_op_type import AluOpType
from concourse.masks import make_identity

F32 = mybir.dt.float32
F32R = mybir.dt.float32r
AF = mybir.ActivationFunctionType

N = 2048
F = 256
D = 128
P = 128
ALPHA = 0.2
NEG = -1.0e30
N_CORES = 8


def build_nc(n=N):
    from concourse import bacc
    nc = bacc.Bacc("TRN2", target_bir_lowering=False, debug=False,
                   enable_asserts=False, num_devices=N_CORES)

    x_d = nc.declare_dram_parameter("x", [n, F], F32, isOutput=False)
    W_d, b_d, a_d = {}, {}, {}
    for l in (0, 1):
        for h in (0, 1):
            W_d[l, h] = nc.declare_dram_parameter(f"W_{l}_{h}", [F, D], F32, isOutput=False)
            b_d[l, h] = nc.declare_dram_parameter(f"b_{l}_{h}", [D], F32, isOutput=False)
            a_d[l, h] = nc.declare_dram_parameter(f"a_{l}_{h}", [2 * D, 1], F32, isOutput=False)
    out_d = nc.declare_dram_parameter("out", [n, F], F32, isOutput=True)

    NJ = n // P          # node chunks of 128 (partition dim of score tiles)
    IB = min(512, n)     # i-block width (moving free dim; 512 = one PSUM bank)
    NI = n // IB

    with tile.TileContext(nc) as tc, ExitStack() as ctx:
        const = ctx.enter_context(tc.tile_pool(name="const", bufs=1))
        persist = ctx.enter_context(tc.tile_pool(name="persist", bufs=1))
        headp = ctx.enter_context(tc.tile_pool(name="headp", bufs=2))
        ztp = ctx.enter_context(tc.tile_pool(name="ztp", bufs=3))
        lrp = ctx.enter_context(tc.tile_pool(name="lrp", bufs=3))
        up = ctx.enter_context(tc.tile_pool(name="up", bufs=4))
        epp = ctx.enter_context(tc.tile_pool(name="epp", bufs=1))
        smallp = ctx.enter_context(tc.tile_pool(name="smallp", bufs=4))
        ps_prep = ctx.enter_context(tc.tile_pool(name="ps_prep", bufs=2, space="PSUM"))
        ps_main = ctx.enter_context(tc.tile_pool(name="ps_main", bufs=1, space="PSUM"))
        ps_z = ctx.enter_context(tc.tile_pool(name="ps_z", bufs=1, space="PSUM"))

        # ---- constants ----
        I128 = const.tile([P, P], F32, tag="I128", name="I128")
        make_identity(nc, I128[:])
        dmask = const.tile([P, P], F32, tag="dmask", name="dmask")
        nc.gpsimd.memset(dmask[:], 0.0)
        nc.gpsimd.affine_select(
            out=dmask[:], in_=dmask[:], compare_op=AluOpType.not_equal,
            fill=NEG, base=0, pattern=[[-1, P]], channel_multiplier=1,
        )
        ones_col_f = const.tile([P, 1], F32, tag="ones_col_f", name="ones_col_f")
        nc.vector.memset(ones_col_f[:], 1.0)
        ones_col = const.tile([P, 2], F32R, tag="ones_col", name="ones_col")
        nc.vector.tensor_copy(ones_col[:], ones_col_f[:].to_broadcast([P, 2]))
        ones_row_f = const.tile([1, P], F32, tag="ones_row_f", name="ones_row_f")
        nc.vector.memset(ones_row_f[:], 1.0)
        ones_row = const.tile([1, P], F32R, tag="ones_row", name="ones_row")
        nc.vector.tensor_copy(ones_row[:], ones_row_f[:])

        # ---- parameters (DMA as f32, one rounding copy into f32r) ----
        Wt, bt, Alt, art = {}, {}, {}, {}
        for l in (0, 1):
            for h in (0, 1):
                Wt[l, h] = []
                for c in range(2):
                    wf = smallp.tile([P, D], F32, tag="wload", name="wload")
                    nc.sync.dma_start(out=wf[:], in_=W_d[l, h][c * P:(c + 1) * P, :])
                    w = const.tile([P, D], F32R, tag=f"W{l}{h}{c}", name=f"W{l}{h}{c}")
                    nc.vector.tensor_copy(w[:], wf[:])
                    Wt[l, h].append(w)
                b = const.tile([P, 1], F32, tag=f"b{l}{h}", name=f"b{l}{h}")
                nc.sync.dma_start(
                    out=b[:], in_=b_d[l, h][:].rearrange("(p o) -> p o", o=1))
                bt[l, h] = b
                # a_l replicated across 128 columns so that matmul(lhsT=Al,
                # rhs=hT) emits hl broadcast across partitions. ACT with
                # scale=0 broadcasts the per-partition bias along free dim.
                alf = smallp.tile([P, 1], F32, tag="alload", name="alload")
                nc.sync.dma_start(out=alf[:], in_=a_d[l, h][0:P, 0:1])
                Al = const.tile([P, P], F32R, tag=f"Al{l}{h}", name=f"Al{l}{h}")
                nc.vector.tensor_copy(Al[:], alf[:].to_broadcast([P, P]))
                Alt[l, h] = Al
                arf = smallp.tile([P, 1], F32, tag="arload", name="arload")
                nc.sync.dma_start(out=arf[:], in_=a_d[l, h][P:2 * P, 0:1])
                ar2 = const.tile([P, 2], F32R, tag=f"ar{l}{h}", name=f"ar{l}{h}")
                nc.vector.tensor_copy(ar2[:], arf[:].to_broadcast([P, 2]))
                art[l, h] = ar2

        # ---- load x and transpose to xT [2 x (P, n)] (f32r: feeds hT-mm) ----
        xT = [persist.tile([P, n], F32R, tag=f"xT{f}", name=f"xT{f}") for f in range(2)]
        for c in range(NJ):
            xc = smallp.tile([P, F], F32, tag="xload", name="xload")
            nc.sync.dma_start(out=xc[:], in_=x_d[c * P:(c + 1) * P, :])
            for f in range(2):
                tp = ps_prep.tile([P, IB], F32, tag="prep", name="prep")
                nc.tensor.transpose(tp[:, 0:P], xc[:, f * P:(f + 1) * P], I128[:])
                if (c + f) % 2 == 0:
                    nc.vector.tensor_copy(xT[f][:, c * P:(c + 1) * P], tp[:, 0:P])
                else:
                    nc.scalar.activation(xT[f][:, c * P:(c + 1) * P], tp[:, 0:P], AF.Copy)

        X1T = [persist.tile([P, n], F32R, tag=f"X1T{f}", name=f"X1T{f}") for f in range(2)]
        X2T = [persist.tile([P, n], F32, tag=f"X2T{f}", name=f"X2T{f}") for f in range(2)]

        def gat_head(XT, Wc, b, Al, ar2, OUT, out_f32r):
            # hT[d, i] = sum_f W[f, d] * xT[f, i]  (+ b via ACT Identity bias)
            hT = headp.tile([P, n], F32R, tag="hT", name="hT")
            hTf = hT[:].bitcast(F32)
            for ib in range(NI):
                sl = slice(ib * IB, (ib + 1) * IB)
                ps = ps_prep.tile([P, IB], F32, tag="prep", name="prep")
                nc.tensor.matmul(ps[:], Wc[0][:], XT[0][:, sl], start=True, stop=False)
                nc.tensor.matmul(ps[:], Wc[1][:], XT[1][:, sl], start=False, stop=True)
                nc.scalar.activation(hT[:, sl], ps[:], AF.Identity, bias=b[:])
            # h chunks [node_part, d_free] via PE transpose (separate tiles to
            # avoid narrow writes into a wide tile)
            hs = []
            for jc in range(NJ):
                tp = ps_prep.tile([P, IB], F32, tag="prep", name="prep")
                nc.tensor.transpose(tp[:, 0:P], hTf[:, jc * P:(jc + 1) * P], I128[:])
                hj = headp.tile([P, P], F32R, tag=f"h{jc}", name=f"h{jc}")
                if jc % 2 == 0:
                    nc.vector.tensor_copy(hj[:], tp[:, 0:P])
                else:
                    nc.scalar.activation(hj[:], tp[:, 0:P], AF.Copy)
                hs.append(hj)
            # hl broadcast across partitions: matmul(Al, hT) -> per-half tiles
            NHalf = max(1, n // (2 * IB))
            HWD = n // NHalf
            hlbs = []
            for hf in range(NHalf):
                hlb_t = headp.tile([P, HWD], F32, tag=f"hlb{hf}", name=f"hlb{hf}")
                for q in range(HWD // IB):
                    sl = slice(hf * HWD + q * IB, hf * HWD + (q + 1) * IB)
                    ps = ps_prep.tile([P, IB], F32, tag="prep", name="prep")
                    nc.tensor.matmul(ps[:], Al[:], hT[:, sl], start=True, stop=True)
                    nc.vector.tensor_copy(hlb_t[:, q * IB:(q + 1) * IB], ps[:])
                hlbs.append(hlb_t)
            # hl/hr in column layout [P, NJ] via per-chunk N=2 matmuls
            psr = ps_prep.tile([P, IB], F32, tag="prep", name="prep")
            for jc in range(NJ):
                nc.tensor.matmul(psr[:, 4 * jc:4 * jc + 2],
                                 hT[:, jc * P:(jc + 1) * P],
                                 Al[:, 0:2], start=True, stop=True)
                nc.tensor.matmul(psr[:, 4 * jc + 2:4 * jc + 4],
                                 hT[:, jc * P:(jc + 1) * P],
                                 ar2[:], start=True, stop=True)
            hlc = headp.tile([P, NJ], F32, tag="hlc", name="hlc")
            hrc = headp.tile([P, NJ], F32, tag="hrc", name="hrc")
            quad = psr[:, 0:4 * NJ].rearrange("p (c t) -> p c t", t=4)
            nc.vector.tensor_copy(hlc[:], quad[:, :, 0])
            nc.vector.tensor_copy(hrc[:], quad[:, :, 2])
            # diagonal correction vector wd = exp(lrelu(hl + hr)) in column
            # layout, then rearranged to a row via small DMAs
            zdc = headp.tile([P, NJ], F32, tag="zdc", name="zdc")
            nc.vector.tensor_tensor(zdc[:], hlc[:], hrc[:], AluOpType.add)
            nc.vector.scalar_tensor_tensor(
                zdc[:], in0=zdc[:], scalar=ALPHA, in1=zdc[:],
                op0=AluOpType.mult, op1=AluOpType.max)
            wdc = headp.tile([P, NJ], F32, tag="wdc", name="wdc")
            nc.scalar.activation(wdc[:], zdc[:], AF.Exp)
            wdrow = persist.tile([1, n], F32, tag="wdrow", name="wdrow")
            for jc in range(NJ):
                nc.sync.dma_start(
                    out=wdrow[0:1, jc * P:(jc + 1) * P],
                    in_=wdc[:, jc:jc + 1])
            wdr = persist.tile([1, n], F32R, tag="wdr", name="wdr")
            nc.vector.tensor_copy(wdr[:], wdrow[:])

            for half in range(NHalf):
                HW_ = HWD
                KH = HW_ // IB
                hsl = slice(half * HW_, (half + 1) * HW_)
                oacc = [ps_main.tile([P, IB], F32, tag=f"oacc{k}", name=f"oacc{k}")
                        for k in range(KH)]
                zacc = [ps_z.tile([2, IB], F32, tag=f"zacc{k}", name=f"zacc{k}")
                        for k in range(KH)]
                for jc in range(NJ):
                    # Split score-tile generation across engines:
                    #   even jc: DVE z-add + DVE leakyrelu (in place)
                    #   odd jc: ACT leakyrelu with hr as per-partition bias
                    u = up.tile([P, HW_], F32R, tag="u", name="u")
                    if jc % 2 == 1:
                        lr = lrp.tile([P, HW_], F32, tag="lr", name="lr")
                        nc.scalar.activation(lr[:], hlbs[half][:], AF.Lrelu,
                                             bias=hrc[:, jc:jc + 1], alpha=ALPHA)
                        nc.scalar.activation(u[:], lr[:], AF.Exp)
                    else:
                        zt = ztp.tile([P, HW_], F32, tag="zt", name="zt")
                        nc.vector.tensor_scalar_add(zt[:], hlbs[half][:],
                                                    hrc[:, jc:jc + 1])
                        nc.vector.scalar_tensor_tensor(
                            zt[:], in0=zt[:], scalar=ALPHA, in1=zt[:],
                            op0=AluOpType.mult, op1=AluOpType.max)
                        nc.scalar.activation(u[:], zt[:], AF.Exp)
                    for k in range(KH):
                        nc.tensor.matmul(oacc[k][:], hs[jc][:],
                                         u[:, k * IB:(k + 1) * IB],
                                         start=(jc == 0), stop=(jc == NJ - 1))
                    for k in range(KH):
                        nc.tensor.matmul(zacc[k][:], ones_col[:],
                                         u[:, k * IB:(k + 1) * IB],
                                         start=(jc == 0), stop=(jc == NJ - 1))
                for k in range(KH):
                    ib = half * KH + k
                    isl = slice(ib * IB, (ib + 1) * IB)
                    # corrected denominator: Z - wd  (diagonal removal)
                    zc = smallp.tile([1, IB], F32, tag="zc", name="zc", bufs=2)
                    nc.vector.tensor_tensor(zc[:], zacc[k][0:1, :],
                                            wdrow[0:1, isl], AluOpType.subtract)
                    recip_f = smallp.tile([1, IB], F32, tag="recip_f", name="recip_f", bufs=2)
                    nc.vector.reciprocal_approx_fast(recip_f[:], zc[:])
                    recip = smallp.tile([1, IB], F32R, tag="recip", name="recip", bufs=2)
                    nc.vector.tensor_copy(recip[:], recip_f[:])
                    rb = ps_prep.tile([P, IB], F32, tag="prep", name="prep")
                    nc.tensor.matmul(rb[:], ones_row[:], recip[:], start=True, stop=True)
                    rbs = epp.tile([P, IB], F32, tag="rbs", name="rbs")
                    nc.scalar.activation(rbs[:], rb[:], AF.Copy)
                    # diagonal numerator correction: oacc -= wd * hT
                    wb = ps_prep.tile([P, IB], F32, tag="prep", name="prep")
                    nc.tensor.matmul(wb[:], ones_row[:], wdr[0:1, isl],
                                     start=True, stop=True)
                    q = epp.tile([P, IB], F32, tag="q", name="q")
                    nc.vector.tensor_tensor(q[:], wb[:], hTf[:, isl], AluOpType.mult)
                    oc = epp.tile([P, IB], F32, tag="oc", name="oc")
                    nc.vector.tensor_tensor(oc[:], oacc[k][:], q[:], AluOpType.subtract)
                    v = epp.tile([P, IB], F32, tag="v", name="v")
                    nc.vector.tensor_tensor(v[:], oc[:], rbs[:], AluOpType.mult)
                    v2 = epp.tile([P, IB], F32, tag="v2", name="v2")
                    nc.vector.tensor_tensor(v2[:], v[:], hTf[:, isl], AluOpType.add)
                    # elu(v2) = relu(v2) + exp(-relu(-v2)) - 1
                    r1 = epp.tile([P, IB], F32, tag="r1", name="r1")
                    nc.scalar.activation(r1[:], v2[:], AF.Relu, scale=-1.0)
                    r3 = epp.tile([P, IB], F32, tag="r3", name="r3")
                    nc.scalar.activation(r3[:], v2[:], AF.Relu)
                    r2 = epp.tile([P, IB], F32, tag="r2", name="r2")
                    nc.scalar.activation(r2[:], r1[:], AF.Exp, scale=-1.0)
                    nc.vector.scalar_tensor_tensor(
                        OUT[:, isl], in0=r2[:], scalar=-1.0, in1=r3[:],
                        op0=AluOpType.add, op1=AluOpType.add)

        gat_head(xT, Wt[0, 0], bt[0, 0], Alt[0, 0], art[0, 0], X1T[0], True)
        gat_head(xT, Wt[0, 1], bt[0, 1], Alt[0, 1], art[0, 1], X1T[1], True)
        gat_head(X1T, Wt[1, 0], bt[1, 0], Alt[1, 0], art[1, 0], X2T[0], False)
        gat_head(X1T, Wt[1, 1], bt[1, 1], Alt[1, 1], art[1, 1], X2T[1], False)

        # ---- transpose X2T back and store ----
        for c in range(NJ):
            ob = smallp.tile([P, F], F32, tag="ob", name="ob")
            for f in range(2):
                tp = ps_prep.tile([P, IB], F32, tag="prep", name="prep")
                nc.tensor.transpose(tp[:, 0:P], X2T[f][:, c * P:(c + 1) * P], I128[:])
                if (c + f) % 2 == 0:
                    nc.vector.tensor_copy(ob[:, f * P:(f + 1) * P], tp[:, 0:P])
                else:
                    nc.scalar.activation(ob[:, f * P:(f + 1) * P], tp[:, 0:P], AF.Copy)
            nc.sync.dma_start(out=out_d[c * P:(c + 1) * P, :], in_=ob[:])

    nc.compile()
    return nc


_CACHE = {}
LAST_RESULTS = None


def kernel(**inputs):
    global LAST_RESULTS
    from concourse.bass_utils import run_bass_kernel_spmd

    x = np.ascontiguousarray(np.asarray(inputs["x"], dtype=np.float32))
    B = x.shape[0]
    assert B == N_CORES and x.shape[1] == N and x.shape[2] == F

    if "nc" not in _CACHE:
        _CACHE["nc"] = build_nc()
    nc = _CACHE["nc"]

    base = {}
    for l in (0, 1):
        for h in (0, 1):
            base[f"W_{l}_{h}"] = np.ascontiguousarray(
                np.asarray(inputs[f"W_{l}_{h}"], dtype=np.float32))
            base[f"b_{l}_{h}"] = np.ascontiguousarray(
                np.asarray(inputs[f"b_{l}_{h}"], dtype=np.float32))
            base[f"a_{l}_{h}"] = np.ascontiguousarray(
                np.asarray(inputs[f"a_{l}_{h}"], dtype=np.float32))

    in_maps = [dict(base, x=np.ascontiguousarray(x[i])) for i in range(B)]
    res = run_bass_kernel_spmd(nc, in_maps, list(range(N_CORES)),
                               trace=bool(os.environ.get("BASS_TRACE")))
    LAST_RESULTS = res
    out = np.stack([res.results[i]["out"] for i in range(B)], axis=0)
    return out.astype(np.float32)


# revision 26
# speedup vs baseline: 2.8837x; 1.1569x over previous
"""Dense 2-layer 2-head GAT for Trainium2 (Bass/Tile), data-parallel over batch.

Each of the 8 NeuronCores processes one batch element (B=8). The per-head
attention score matrix s[i,j] = leakyrelu(hl_i + hr_j) is rank-1 structured,
so score tiles are generated on-chip (never materialized in DRAM):

  - hl broadcast across partitions comes from a single matmul with a
    column-replicated `a_l` stationary operand against hT.
  - hr enters as a per-partition scalar (DVE tensor_scalar add, or fused as
    the per-partition bias of an ACT Lrelu, or on GPSIMD) - score-tile
    generation is statically spread across DVE/ACT/GPSIMD to balance engines.
  - exp on the scalar engine (ACT), written as float32r so the TensorE
    consumes it at 1 cyc/col.
  - p @ h and the softmax denominator are fp32r matmuls.
  - The diagonal mask is an additive -1e30 eye stripe on the tiles that touch
    the diagonal; softmax runs without max-subtraction (scores bounded ~11,
    exp <= 5e4, fp32-safe; the unnormalized ratio is shift-invariant so
    results match the reference's max-subtracted softmax).

Everything stays in the transposed layout [feat_part, node_free] so each
layer's output feeds the next layer's matmul directly; only the initial x
load and final store transpose via the PE.

Tiles consumed by fp32r matmuls are allocated as float32r and written by ops
directly in that dtype (the BIR verifier requires producers to round);
DVE/ACT consumers of those tiles read them via a bitcast-to-f32 view.
"""

import os
from contextlib import ExitStack

import numpy as np

import concourse.bass as bass
import concourse.mybir as mybir
import concourse.tile as tile
from concourse.alu_op_type import AluOpType
from concourse.masks import make_identity

F32 = mybir.dt.float32
F32R = mybir.dt.float32r
AF = mybir.ActivationFunctionType

N = 2048
F = 256
D = 128
P = 128
ALPHA = 0.2
NEG = -1.0e30
N_CORES = 8


def build_nc(n=N):
    from concourse import bacc
    nc = bacc.Bacc("TRN2", target_bir_lowering=False, debug=False,
                   enable_asserts=False, num_devices=N_CORES)

    x_d = nc.declare_dram_parameter("x", [n, F], F32, isOutput=False)
    W_d, b_d, a_d = {}, {}, {}
    for l in (0, 1):
        for h in (0, 1):
            W_d[l, h] = nc.declare_dram_parameter(f"W_{l}_{h}", [F, D], F32, isOutput=False)
            b_d[l, h] = nc.declare_dram_parameter(f"b_{l}_{h}", [D], F32, isOutput=False)
            a_d[l, h] = nc.declare_dram_parameter(f"a_{l}_{h}", [2 * D, 1], F32, isOutput=False)
    out_d = nc.declare_dram_parameter("out", [n, F], F32, isOutput=True)

    NJ = n // P          # node chunks of 128 (partition dim of score tiles)
    IB = min(512, n)     # i-block width (moving free dim; 512 = one PSUM bank)
    NI = n // IB

    with tile.TileContext(nc) as tc, ExitStack() as ctx:
        const = ctx.enter_context(tc.tile_pool(name="const", bufs=1))
        persist = ctx.enter_context(tc.tile_pool(name="persist", bufs=1))
        headp = ctx.enter_context(tc.tile_pool(name="headp", bufs=2))
        ztp = ctx.enter_context(tc.tile_pool(name="ztp", bufs=3))
        lrp = ctx.enter_context(tc.tile_pool(name="lrp", bufs=3))
        up = ctx.enter_context(tc.tile_pool(name="up", bufs=4))
        epp = ctx.enter_context(tc.tile_pool(name="epp", bufs=1))
        smallp = ctx.enter_context(tc.tile_pool(name="smallp", bufs=4))
        ps_prep = ctx.enter_context(tc.tile_pool(name="ps_prep", bufs=2, space="PSUM"))
        ps_main = ctx.enter_context(tc.tile_pool(name="ps_main", bufs=2, space="PSUM"))
        ps_z = ctx.enter_context(tc.tile_pool(name="ps_z", bufs=1, space="PSUM"))

        # ---- constants ----
        I128 = const.tile([P, P], F32, tag="I128", name="I128")
        make_identity(nc, I128[:])
        dmask = const.tile([P, P], F32, tag="dmask", name="dmask")
        nc.gpsimd.memset(dmask[:], 0.0)
        nc.gpsimd.affine_select(
            out=dmask[:], in_=dmask[:], compare_op=AluOpType.not_equal,
            fill=NEG, base=0, pattern=[[-1, P]], channel_multiplier=1,
        )
        ones_col_f = const.tile([P, 1], F32, tag="ones_col_f", name="ones_col_f")
        nc.vector.memset(ones_col_f[:], 1.0)
        ones_col = const.tile([P, 2], F32R, tag="ones_col", name="ones_col")
        nc.vector.tensor_copy(ones_col[:], ones_col_f[:].to_broadcast([P, 2]))
        ones_row_f = const.tile([1, P], F32, tag="ones_row_f", name="ones_row_f")
        nc.vector.memset(ones_row_f[:], 1.0)
        ones_row = const.tile([1, P], F32R, tag="ones_row", name="ones_row")
        nc.vector.tensor_copy(ones_row[:], ones_row_f[:])

        # ---- parameters (DMA as f32, one rounding copy into f32r) ----
        Wt, bt, Alt, art = {}, {}, {}, {}
        for l in (0, 1):
            for h in (0, 1):
                Wt[l, h] = []
                for c in range(2):
                    wf = smallp.tile([P, D], F32, tag="wload", name="wload")
                    nc.sync.dma_start(out=wf[:], in_=W_d[l, h][c * P:(c + 1) * P, :])
                    w = const.tile([P, D], F32R, tag=f"W{l}{h}{c}", name=f"W{l}{h}{c}")
                    nc.vector.tensor_copy(w[:], wf[:])
                    Wt[l, h].append(w)
                b = const.tile([P, 1], F32, tag=f"b{l}{h}", name=f"b{l}{h}")
                nc.sync.dma_start(
                    out=b[:], in_=b_d[l, h][:].rearrange("(p o) -> p o", o=1))
                bt[l, h] = b
                # a_l replicated across 128 columns so that matmul(lhsT=Al,
                # rhs=hT) emits hl broadcast across partitions. ACT with
                # scale=0 broadcasts the per-partition bias along free dim.
                alf = smallp.tile([P, 1], F32, tag="alload", name="alload")
                nc.sync.dma_start(out=alf[:], in_=a_d[l, h][0:P, 0:1])
                Al = const.tile([P, P], F32R, tag=f"Al{l}{h}", name=f"Al{l}{h}")
                nc.vector.tensor_copy(Al[:], alf[:].to_broadcast([P, P]))
                Alt[l, h] = Al
                arf = smallp.tile([P, 1], F32, tag="arload", name="arload")
                nc.sync.dma_start(out=arf[:], in_=a_d[l, h][P:2 * P, 0:1])
                ar2 = const.tile([P, 2], F32R, tag=f"ar{l}{h}", name=f"ar{l}{h}")
                nc.vector.tensor_copy(ar2[:], arf[:].to_broadcast([P, 2]))
                art[l, h] = ar2

        # ---- load x and transpose to xT [2 x (P, n)] (f32r: feeds hT-mm) ----
        xT = [persist.tile([P, n], F32R, tag=f"xT{f}", name=f"xT{f}") for f in range(2)]
        for c in range(NJ):
            xc = smallp.tile([P, F], F32, tag="xload", name="xload")
            nc.sync.dma_start(out=xc[:], in_=x_d[c * P:(c + 1) * P, :])
            for f in range(2):
                tp = ps_prep.tile([P, IB], F32, tag="prep", name="prep")
                nc.tensor.transpose(tp[:, 0:P], xc[:, f * P:(f + 1) * P], I128[:])
                if (c + f) % 2 == 0:
                    nc.vector.tensor_copy(xT[f][:, c * P:(c + 1) * P], tp[:, 0:P])
                else:
                    nc.scalar.activation(xT[f][:, c * P:(c + 1) * P], tp[:, 0:P], AF.Copy)

        X1T = [persist.tile([P, n], F32R, tag=f"X1T{f}", name=f"X1T{f}") for f in range(2)]
        X2T = [persist.tile([P, n], F32, tag=f"X2T{f}", name=f"X2T{f}") for f in range(2)]

        def gat_head(XT, Wc, b, Al, ar2, OUT, out_f32r):
            # hT[d, i] = sum_f W[f, d] * xT[f, i]  (+ b via ACT Identity bias)
            hT = headp.tile([P, n], F32R, tag="hT", name="hT")
            hTf = hT[:].bitcast(F32)
            for ib in range(NI):
                sl = slice(ib * IB, (ib + 1) * IB)
                ps = ps_prep.tile([P, IB], F32, tag="prep", name="prep")
                nc.tensor.matmul(ps[:], Wc[0][:], XT[0][:, sl], start=True, stop=False)
                nc.tensor.matmul(ps[:], Wc[1][:], XT[1][:, sl], start=False, stop=True)
                nc.scalar.activation(hT[:, sl], ps[:], AF.Identity, bias=b[:])
            # h chunks [node_part, d_free] via PE transpose (separate tiles to
            # avoid narrow writes into a wide tile)
            hs = []
            for jc in range(NJ):
                tp = ps_prep.tile([P, IB], F32, tag="prep", name="prep")
                nc.tensor.transpose(tp[:, 0:P], hTf[:, jc * P:(jc + 1) * P], I128[:])
                hj = headp.tile([P, P], F32R, tag=f"h{jc}", name=f"h{jc}")
                if jc % 2 == 0:
                    nc.vector.tensor_copy(hj[:], tp[:, 0:P])
                else:
                    nc.scalar.activation(hj[:], tp[:, 0:P], AF.Copy)
                hs.append(hj)
            # hl broadcast across partitions: matmul(Al, hT) -> per-half tiles
            NHalf = max(1, n // (2 * IB))
            HWD = n // NHalf
            hlbs = []
            for hf in range(NHalf):
                hlb_t = headp.tile([P, HWD], F32, tag=f"hlb{hf}", name=f"hlb{hf}")
                for q in range(HWD // IB):
                    sl = slice(hf * HWD + q * IB, hf * HWD + (q + 1) * IB)
                    ps = ps_prep.tile([P, IB], F32, tag="prep", name="prep")
                    nc.tensor.matmul(ps[:], Al[:], hT[:, sl], start=True, stop=True)
                    nc.vector.tensor_copy(hlb_t[:, q * IB:(q + 1) * IB], ps[:])
                hlbs.append(hlb_t)
            # hl/hr in column layout [P, NJ] via per-chunk N=2 matmuls
            psr = ps_prep.tile([P, IB], F32, tag="prep", name="prep")
            for jc in range(NJ):
                nc.tensor.matmul(psr[:, 4 * jc:4 * jc + 2],
                                 hT[:, jc * P:(jc + 1) * P],
                                 Al[:, 0:2], start=True, stop=True)
                nc.tensor.matmul(psr[:, 4 * jc + 2:4 * jc + 4],
                                 hT[:, jc * P:(jc + 1) * P],
                                 ar2[:], start=True, stop=True)
            hlc = headp.tile([P, NJ], F32, tag="hlc", name="hlc")
            hrc = headp.tile([P, NJ], F32, tag="hrc", name="hrc")
            quad = psr[:, 0:4 * NJ].rearrange("p (c t) -> p c t", t=4)
            nc.vector.tensor_copy(hlc[:], quad[:, :, 0])
            nc.vector.tensor_copy(hrc[:], quad[:, :, 2])
            # diagonal correction vector wd = exp(lrelu(hl + hr)) in column
            # layout, then rearranged to a row via small DMAs
            zdc = headp.tile([P, NJ], F32, tag="zdc", name="zdc")
            nc.vector.tensor_tensor(zdc[:], hlc[:], hrc[:], AluOpType.add)
            nc.vector.scalar_tensor_tensor(
                zdc[:], in0=zdc[:], scalar=ALPHA, in1=zdc[:],
                op0=AluOpType.mult, op1=AluOpType.max)
            wdc = headp.tile([P, NJ], F32, tag="wdc", name="wdc")
            nc.scalar.activation(wdc[:], zdc[:], AF.Exp)
            wdrow = persist.tile([1, n], F32, tag="wdrow", name="wdrow")
            for jc in range(NJ):
                nc.sync.dma_start(
                    out=wdrow[0:1, jc * P:(jc + 1) * P],
                    in_=wdc[:, jc:jc + 1])
            wdr = persist.tile([1, n], F32R, tag="wdr", name="wdr")
            nc.vector.tensor_copy(wdr[:], wdrow[:])

            for half in range(NHalf):
                HW_ = HWD
                KH = HW_ // IB
                hsl = slice(half * HW_, (half + 1) * HW_)
                oacc = [ps_main.tile([P, IB], F32, tag=f"oacc{k}", name=f"oacc{k}")
                        for k in range(KH)]
                zacc = [ps_z.tile([2, IB], F32, tag=f"zacc{k}", name=f"zacc{k}")
                        for k in range(KH)]
                for jc in range(NJ):
                    # Split score-tile generation across engines:
                    #   even jc: DVE z-add + DVE leakyrelu (in place)
                    #   odd jc: ACT leakyrelu with hr as per-partition bias
                    u = up.tile([P, HW_], F32R, tag="u", name="u")
                    if jc % 4 == 1:
                        lr = lrp.tile([P, HW_], F32, tag="lr", name="lr")
                        nc.scalar.activation(lr[:], hlbs[half][:], AF.Lrelu,
                                             bias=hrc[:, jc:jc + 1], alpha=ALPHA)
                        nc.scalar.activation(u[:], lr[:], AF.Exp)
                    else:
                        zt = ztp.tile([P, HW_], F32, tag="zt", name="zt")
                        nc.vector.tensor_scalar_add(zt[:], hlbs[half][:],
                                                    hrc[:, jc:jc + 1])
                        nc.vector.scalar_tensor_tensor(
                            zt[:], in0=zt[:], scalar=ALPHA, in1=zt[:],
                            op0=AluOpType.mult, op1=AluOpType.max)
                        nc.scalar.activation(u[:], zt[:], AF.Exp)
                    for k in range(KH):
                        nc.tensor.matmul(oacc[k][:], hs[jc][:],
                                         u[:, k * IB:(k + 1) * IB],
                                         start=(jc == 0), stop=(jc == NJ - 1))
                    for k in range(KH):
                        nc.tensor.matmul(zacc[k][:], ones_col[:],
                                         u[:, k * IB:(k + 1) * IB],
                                         start=(jc == 0), stop=(jc == NJ - 1))
                for k in range(KH):
                    ib = half * KH + k
                    isl = slice(ib * IB, (ib + 1) * IB)
                    # corrected denominator: Z - wd  (diagonal removal)
                    zc = smallp.tile([1, IB], F32, tag="zc", name="zc", bufs=2)
                    nc.vector.tensor_tensor(zc[:], zacc[k][0:1, :],
                                            wdrow[0:1, isl], AluOpType.subtract)
                    recip_f = smallp.tile([1, IB], F32, tag="recip_f", name="recip_f", bufs=2)
                    nc.vector.reciprocal_approx_fast(recip_f[:], zc[:])
                    recip = smallp.tile([1, IB], F32R, tag="recip", name="recip", bufs=2)
                    nc.vector.tensor_copy(recip[:], recip_f[:])
                    rb = ps_prep.tile([P, IB], F32, tag="prep", name="prep")
                    nc.tensor.matmul(rb[:], ones_row[:], recip[:], start=True, stop=True)
                    rbs = epp.tile([P, IB], F32, tag="rbs", name="rbs")
                    nc.scalar.activation(rbs[:], rb[:], AF.Copy)
                    # diagonal numerator correction: oacc -= wd * hT
                    wb = ps_prep.tile([P, IB], F32, tag="prep", name="prep")
                    nc.tensor.matmul(wb[:], ones_row[:], wdr[0:1, isl],
                                     start=True, stop=True)
                    q = epp.tile([P, IB], F32, tag="q", name="q")
                    nc.vector.tensor_tensor(q[:], wb[:], hTf[:, isl], AluOpType.mult)
                    oc = epp.tile([P, IB], F32, tag="oc", name="oc")
                    nc.vector.tensor_tensor(oc[:], oacc[k][:], q[:], AluOpType.subtract)
                    v = epp.tile([P, IB], F32, tag="v", name="v")
                    nc.vector.tensor_tensor(v[:], oc[:], rbs[:], AluOpType.mult)
                    v2 = epp.tile([P, IB], F32, tag="v2", name="v2")
                    nc.vector.tensor_tensor(v2[:], v[:], hTf[:, isl], AluOpType.add)
                    # elu(v2) = relu(v2) + exp(-relu(-v2)) - 1
                    r1 = epp.tile([P, IB], F32, tag="r1", name="r1")
                    nc.scalar.activation(r1[:], v2[:], AF.Relu, scale=-1.0)
                    r3 = epp.tile([P, IB], F32, tag="r3", name="r3")
                    nc.scalar.activation(r3[:], v2[:], AF.Relu)
                    r2 = epp.tile([P, IB], F32, tag="r2", name="r2")
                    nc.scalar.activation(r2[:], r1[:], AF.Exp, scale=-1.0)
                    nc.vector.scalar_tensor_tensor(
                        OUT[:, isl], in0=r2[:], scalar=-1.0, in1=r3[:],
                        op0=AluOpType.add, op1=AluOpType.add)

        gat_head(xT, Wt[0, 0], bt[0, 0], Alt[0, 0], art[0, 0], X1T[0], True)
        gat_head(xT, Wt[0, 1], bt[0, 1], Alt[0, 1], art[0, 1], X1T[1], True)
        gat_head(X1T, Wt[1, 0], bt[1, 0], Alt[1, 0], art[1, 0], X2T[0], False)
        gat_head(X1T, Wt[1, 1], bt[1, 1], Alt[1, 1], art[1, 1], X2T[1], False)

        # ---- transpose X2T back and store ----
        for c in range(NJ):
            ob = smallp.tile([P, F], F32, tag="ob", name="ob")
            for f in range(2):
                tp = ps_prep.tile([P, IB], F32, tag="prep", name="prep")
                nc.tensor.transpose(tp[:, 0:P], X2T[f][:, c * P:(c + 1) * P], I128[:])
                if (c + f) % 2 == 0:
                    nc.vector.tensor_copy(ob[:, f * P:(f + 1) * P], tp[:, 0:P])
                else:
                    nc.scalar.activation(ob[:, f * P:(f + 1) * P], tp[:, 0:P], AF.Copy)
            nc.sync.dma_start(out=out_d[c * P:(c + 1) * P, :], in_=ob[:])

    nc.compile()
    return nc


_CACHE = {}
LAST_RESULTS = None


def kernel(**inputs):
    global LAST_RESULTS
    from concourse.bass_utils import run_bass_kernel_spmd

    x = np.ascontiguousarray(np.asarray(inputs["x"], dtype=np.float32))
    B = x.shape[0]
    assert B == N_CORES and x.shape[1] == N and x.shape[2] == F

    if "nc" not in _CACHE:
        _CACHE["nc"] = build_nc()
    nc = _CACHE["nc"]

    base = {}
    for l in (0, 1):
        for h in (0, 1):
            base[f"W_{l}_{h}"] = np.ascontiguousarray(
                np.asarray(inputs[f"W_{l}_{h}"], dtype=np.float32))
            base[f"b_{l}_{h}"] = np.ascontiguousarray(
                np.asarray(inputs[f"b_{l}_{h}"], dtype=np.float32))
            base[f"a_{l}_{h}"] = np.ascontiguousarray(
                np.asarray(inputs[f"a_{l}_{h}"], dtype=np.float32))

    in_maps = [dict(base, x=np.ascontiguousarray(x[i])) for i in range(B)]
    res = run_bass_kernel_spmd(nc, in_maps, list(range(N_CORES)),
                               trace=bool(os.environ.get("BASS_TRACE")))
    LAST_RESULTS = res
    out = np.stack([res.results[i]["out"] for i in range(B)], axis=0)
    return out.astype(np.float32)


# revision 27
# speedup vs baseline: 2.9041x; 1.0071x over previous
"""Dense 2-layer 2-head GAT for Trainium2 (Bass/Tile), data-parallel over batch.

Each of the 8 NeuronCores processes one batch element (B=8). The per-head
attention score matrix s[i,j] = leakyrelu(hl_i + hr_j) is rank-1 structured,
so score tiles are generated on-chip (never materialized in DRAM):

  - hl broadcast across partitions comes from a single matmul with a
    column-replicated `a_l` stationary operand against hT.
  - hr enters as a per-partition scalar (DVE tensor_scalar add, or fused as
    the per-partition bias of an ACT Lrelu, or on GPSIMD) - score-tile
    generation is statically spread across DVE/ACT/GPSIMD to balance engines.
  - exp on the scalar engine (ACT), written as float32r so the TensorE
    consumes it at 1 cyc/col.
  - p @ h and the softmax denominator are fp32r matmuls.
  - The diagonal mask is an additive -1e30 eye stripe on the tiles that touch
    the diagonal; softmax runs without max-subtraction (scores bounded ~11,
    exp <= 5e4, fp32-safe; the unnormalized ratio is shift-invariant so
    results match the reference's max-subtracted softmax).

Everything stays in the transposed layout [feat_part, node_free] so each
layer's output feeds the next layer's matmul directly; only the initial x
load and final store transpose via the PE.

Tiles consumed by fp32r matmuls are allocated as float32r and written by ops
directly in that dtype (the BIR verifier requires producers to round);
DVE/ACT consumers of those tiles read them via a bitcast-to-f32 view.
"""

import os
from contextlib import ExitStack

import numpy as np

import concourse.bass as bass
import concourse.mybir as mybir
import concourse.tile as tile
from concourse.alu_op_type import AluOpType
from concourse.masks import make_identity

F32 = mybir.dt.float32
F32R = mybir.dt.float32r
AF = mybir.ActivationFunctionType

N = 2048
F = 256
D = 128
P = 128
ALPHA = 0.2
NEG = -1.0e30
N_CORES = 8


def build_nc(n=N):
    from concourse import bacc
    nc = bacc.Bacc("TRN2", target_bir_lowering=False, debug=False,
                   enable_asserts=False, num_devices=N_CORES)

    x_d = nc.declare_dram_parameter("x", [n, F], F32, isOutput=False)
    W_d, b_d, a_d = {}, {}, {}
    for l in (0, 1):
        for h in (0, 1):
            W_d[l, h] = nc.declare_dram_parameter(f"W_{l}_{h}", [F, D], F32, isOutput=False)
            b_d[l, h] = nc.declare_dram_parameter(f"b_{l}_{h}", [D], F32, isOutput=False)
            a_d[l, h] = nc.declare_dram_parameter(f"a_{l}_{h}", [2 * D, 1], F32, isOutput=False)
    out_d = nc.declare_dram_parameter("out", [n, F], F32, isOutput=True)

    NJ = n // P          # node chunks of 128 (partition dim of score tiles)
    IB = min(512, n)     # i-block width (moving free dim; 512 = one PSUM bank)
    NI = n // IB

    with tile.TileContext(nc) as tc, ExitStack() as ctx:
        const = ctx.enter_context(tc.tile_pool(name="const", bufs=1))
        persist = ctx.enter_context(tc.tile_pool(name="persist", bufs=1))
        headp = ctx.enter_context(tc.tile_pool(name="headp", bufs=2))
        ztp = ctx.enter_context(tc.tile_pool(name="ztp", bufs=3))
        lrp = ctx.enter_context(tc.tile_pool(name="lrp", bufs=3))
        up = ctx.enter_context(tc.tile_pool(name="up", bufs=5))
        epp = ctx.enter_context(tc.tile_pool(name="epp", bufs=1))
        smallp = ctx.enter_context(tc.tile_pool(name="smallp", bufs=4))
        ps_prep = ctx.enter_context(tc.tile_pool(name="ps_prep", bufs=2, space="PSUM"))
        ps_main = ctx.enter_context(tc.tile_pool(name="ps_main", bufs=2, space="PSUM"))
        ps_z = ctx.enter_context(tc.tile_pool(name="ps_z", bufs=1, space="PSUM"))

        # ---- constants ----
        I128 = const.tile([P, P], F32, tag="I128", name="I128")
        make_identity(nc, I128[:])
        dmask = const.tile([P, P], F32, tag="dmask", name="dmask")
        nc.gpsimd.memset(dmask[:], 0.0)
        nc.gpsimd.affine_select(
            out=dmask[:], in_=dmask[:], compare_op=AluOpType.not_equal,
            fill=NEG, base=0, pattern=[[-1, P]], channel_multiplier=1,
        )
        ones_col_f = const.tile([P, 1], F32, tag="ones_col_f", name="ones_col_f")
        nc.vector.memset(ones_col_f[:], 1.0)
        ones_col = const.tile([P, 2], F32R, tag="ones_col", name="ones_col")
        nc.vector.tensor_copy(ones_col[:], ones_col_f[:].to_broadcast([P, 2]))
        ones_row_f = const.tile([1, P], F32, tag="ones_row_f", name="ones_row_f")
        nc.vector.memset(ones_row_f[:], 1.0)
        ones_row = const.tile([1, P], F32R, tag="ones_row", name="ones_row")
        nc.vector.tensor_copy(ones_row[:], ones_row_f[:])

        # ---- parameters (DMA as f32, one rounding copy into f32r) ----
        Wt, bt, Alt, art = {}, {}, {}, {}
        for l in (0, 1):
            for h in (0, 1):
                Wt[l, h] = []
                for c in range(2):
                    wf = smallp.tile([P, D], F32, tag="wload", name="wload")
                    nc.sync.dma_start(out=wf[:], in_=W_d[l, h][c * P:(c + 1) * P, :])
                    w = const.tile([P, D], F32R, tag=f"W{l}{h}{c}", name=f"W{l}{h}{c}")
                    nc.vector.tensor_copy(w[:], wf[:])
                    Wt[l, h].append(w)
                b = const.tile([P, 1], F32, tag=f"b{l}{h}", name=f"b{l}{h}")
                nc.sync.dma_start(
                    out=b[:], in_=b_d[l, h][:].rearrange("(p o) -> p o", o=1))
                bt[l, h] = b
                # a_l replicated across 128 columns so that matmul(lhsT=Al,
                # rhs=hT) emits hl broadcast across partitions. ACT with
                # scale=0 broadcasts the per-partition bias along free dim.
                alf = smallp.tile([P, 1], F32, tag="alload", name="alload")
                nc.sync.dma_start(out=alf[:], in_=a_d[l, h][0:P, 0:1])
                Al = const.tile([P, P], F32R, tag=f"Al{l}{h}", name=f"Al{l}{h}")
                nc.vector.tensor_copy(Al[:], alf[:].to_broadcast([P, P]))
                Alt[l, h] = Al
                arf = smallp.tile([P, 1], F32, tag="arload", name="arload")
                nc.sync.dma_start(out=arf[:], in_=a_d[l, h][P:2 * P, 0:1])
                ar2 = const.tile([P, 2], F32R, tag=f"ar{l}{h}", name=f"ar{l}{h}")
                nc.vector.tensor_copy(ar2[:], arf[:].to_broadcast([P, 2]))
                art[l, h] = ar2

        # ---- load x and transpose to xT [2 x (P, n)] (f32r: feeds hT-mm) ----
        xT = [persist.tile([P, n], F32R, tag=f"xT{f}", name=f"xT{f}") for f in range(2)]
        for c in range(NJ):
            xc = smallp.tile([P, F], F32, tag="xload", name="xload")
            nc.sync.dma_start(out=xc[:], in_=x_d[c * P:(c + 1) * P, :])
            for f in range(2):
                tp = ps_prep.tile([P, IB], F32, tag="prep", name="prep")
                nc.tensor.transpose(tp[:, 0:P], xc[:, f * P:(f + 1) * P], I128[:])
                if (c + f) % 2 == 0:
                    nc.vector.tensor_copy(xT[f][:, c * P:(c + 1) * P], tp[:, 0:P])
                else:
                    nc.scalar.activation(xT[f][:, c * P:(c + 1) * P], tp[:, 0:P], AF.Copy)

        X1T = [persist.tile([P, n], F32R, tag=f"X1T{f}", name=f"X1T{f}") for f in range(2)]
        X2T = [persist.tile([P, n], F32, tag=f"X2T{f}", name=f"X2T{f}") for f in range(2)]

        def gat_head(XT, Wc, b, Al, ar2, OUT, out_f32r):
            # hT[d, i] = sum_f W[f, d] * xT[f, i]  (+ b via ACT Identity bias)
            hT = headp.tile([P, n], F32R, tag="hT", name="hT")
            hTf = hT[:].bitcast(F32)
            for ib in range(NI):
                sl = slice(ib * IB, (ib + 1) * IB)
                ps = ps_prep.tile([P, IB], F32, tag="prep", name="prep")
                nc.tensor.matmul(ps[:], Wc[0][:], XT[0][:, sl], start=True, stop=False)
                nc.tensor.matmul(ps[:], Wc[1][:], XT[1][:, sl], start=False, stop=True)
                nc.vector.tensor_scalar_add(hT[:, sl], ps[:], b[:])
            # h chunks [node_part, d_free] via PE transpose (separate tiles to
            # avoid narrow writes into a wide tile)
            hs = []
            for jc in range(NJ):
                tp = ps_prep.tile([P, IB], F32, tag="prep", name="prep")
                nc.tensor.transpose(tp[:, 0:P], hTf[:, jc * P:(jc + 1) * P], I128[:])
                hj = headp.tile([P, P], F32R, tag=f"h{jc}", name=f"h{jc}")
                if jc % 2 == 0:
                    nc.vector.tensor_copy(hj[:], tp[:, 0:P])
                else:
                    nc.scalar.activation(hj[:], tp[:, 0:P], AF.Copy)
                hs.append(hj)
            # hl broadcast across partitions: matmul(Al, hT) -> per-half tiles
            NHalf = max(1, n // (2 * IB))
            HWD = n // NHalf
            hlbs = []
            for hf in range(NHalf):
                hlb_t = headp.tile([P, HWD], F32, tag=f"hlb{hf}", name=f"hlb{hf}")
                for q in range(HWD // IB):
                    sl = slice(hf * HWD + q * IB, hf * HWD + (q + 1) * IB)
                    ps = ps_prep.tile([P, IB], F32, tag="prep", name="prep")
                    nc.tensor.matmul(ps[:], Al[:], hT[:, sl], start=True, stop=True)
                    nc.vector.tensor_copy(hlb_t[:, q * IB:(q + 1) * IB], ps[:])
                hlbs.append(hlb_t)
            # hl/hr in column layout [P, NJ] via per-chunk N=2 matmuls
            psr = ps_prep.tile([P, IB], F32, tag="prep", name="prep")
            for jc in range(NJ):
                nc.tensor.matmul(psr[:, 4 * jc:4 * jc + 2],
                                 hT[:, jc * P:(jc + 1) * P],
                                 Al[:, 0:2], start=True, stop=True)
                nc.tensor.matmul(psr[:, 4 * jc + 2:4 * jc + 4],
                                 hT[:, jc * P:(jc + 1) * P],
                                 ar2[:], start=True, stop=True)
            hlc = headp.tile([P, NJ], F32, tag="hlc", name="hlc")
            hrc = headp.tile([P, NJ], F32, tag="hrc", name="hrc")
            quad = psr[:, 0:4 * NJ].rearrange("p (c t) -> p c t", t=4)
            nc.vector.tensor_copy(hlc[:], quad[:, :, 0])
            nc.vector.tensor_copy(hrc[:], quad[:, :, 2])
            # diagonal correction vector wd = exp(lrelu(hl + hr)) in column
            # layout, then rearranged to a row via small DMAs
            zdc = headp.tile([P, NJ], F32, tag="zdc", name="zdc")
            nc.vector.tensor_tensor(zdc[:], hlc[:], hrc[:], AluOpType.add)
            nc.vector.scalar_tensor_tensor(
                zdc[:], in0=zdc[:], scalar=ALPHA, in1=zdc[:],
                op0=AluOpType.mult, op1=AluOpType.max)
            wdc = headp.tile([P, NJ], F32, tag="wdc", name="wdc")
            nc.scalar.activation(wdc[:], zdc[:], AF.Exp)
            wdrow = persist.tile([1, n], F32, tag="wdrow", name="wdrow")
            for jc in range(NJ):
                nc.sync.dma_start(
                    out=wdrow[0:1, jc * P:(jc + 1) * P],
                    in_=wdc[:, jc:jc + 1])
            wdr = persist.tile([1, n], F32R, tag="wdr", name="wdr")
            nc.vector.tensor_copy(wdr[:], wdrow[:])

            for half in range(NHalf):
                HW_ = HWD
                KH = HW_ // IB
                hsl = slice(half * HW_, (half + 1) * HW_)
                oacc = [ps_main.tile([P, IB], F32, tag=f"oacc{k}", name=f"oacc{k}")
                        for k in range(KH)]
                zacc = [ps_z.tile([2, IB], F32, tag=f"zacc{k}", name=f"zacc{k}")
                        for k in range(KH)]
                for jc in range(NJ):
                    # Split score-tile generation across engines:
                    #   even jc: DVE z-add + DVE leakyrelu (in place)
                    #   odd jc: ACT leakyrelu with hr as per-partition bias
                    u = up.tile([P, HW_], F32R, tag="u", name="u")
                    if jc % 4 == 1:
                        lr = lrp.tile([P, HW_], F32, tag="lr", name="lr")
                        nc.scalar.activation(lr[:], hlbs[half][:], AF.Lrelu,
                                             bias=hrc[:, jc:jc + 1], alpha=ALPHA)
                        nc.scalar.activation(u[:], lr[:], AF.Exp)
                    else:
                        zt = ztp.tile([P, HW_], F32, tag="zt", name="zt")
                        nc.vector.tensor_scalar_add(zt[:], hlbs[half][:],
                                                    hrc[:, jc:jc + 1])
                        nc.vector.scalar_tensor_tensor(
                            zt[:], in0=zt[:], scalar=ALPHA, in1=zt[:],
                            op0=AluOpType.mult, op1=AluOpType.max)
                        nc.scalar.activation(u[:], zt[:], AF.Exp)
                    for k in range(KH):
                        nc.tensor.matmul(oacc[k][:], hs[jc][:],
                                         u[:, k * IB:(k + 1) * IB],
                                         start=(jc == 0), stop=(jc == NJ - 1))
                    for k in range(KH):
                        nc.tensor.matmul(zacc[k][:], ones_col[:],
                                         u[:, k * IB:(k + 1) * IB],
                                         start=(jc == 0), stop=(jc == NJ - 1))
                for k in range(KH):
                    ib = half * KH + k
                    isl = slice(ib * IB, (ib + 1) * IB)
                    # corrected denominator: Z - wd  (diagonal removal)
                    zc = smallp.tile([1, IB], F32, tag="zc", name="zc", bufs=2)
                    nc.vector.tensor_tensor(zc[:], zacc[k][0:1, :],
                                            wdrow[0:1, isl], AluOpType.subtract)
                    recip_f = smallp.tile([1, IB], F32, tag="recip_f", name="recip_f", bufs=2)
                    nc.vector.reciprocal_approx_fast(recip_f[:], zc[:])
                    recip = smallp.tile([1, IB], F32R, tag="recip", name="recip", bufs=2)
                    nc.vector.tensor_copy(recip[:], recip_f[:])
                    rb = ps_prep.tile([P, IB], F32, tag="prep", name="prep")
                    nc.tensor.matmul(rb[:], ones_row[:], recip[:], start=True, stop=True)
                    rbs = epp.tile([P, IB], F32, tag="rbs", name="rbs")
                    nc.vector.tensor_copy(rbs[:], rb[:])
                    # diagonal numerator correction: oacc -= wd * hT
                    wb = ps_prep.tile([P, IB], F32, tag="prep", name="prep")
                    nc.tensor.matmul(wb[:], ones_row[:], wdr[0:1, isl],
                                     start=True, stop=True)
                    q = epp.tile([P, IB], F32, tag="q", name="q")
                    nc.vector.tensor_tensor(q[:], wb[:], hTf[:, isl], AluOpType.mult)
                    oc = epp.tile([P, IB], F32, tag="oc", name="oc")
                    nc.vector.tensor_tensor(oc[:], oacc[k][:], q[:], AluOpType.subtract)
                    v = epp.tile([P, IB], F32, tag="v", name="v")
                    nc.vector.tensor_tensor(v[:], oc[:], rbs[:], AluOpType.mult)
                    v2 = epp.tile([P, IB], F32, tag="v2", name="v2")
                    nc.vector.tensor_tensor(v2[:], v[:], hTf[:, isl], AluOpType.add)
                    # elu(v2) = relu(v2) + exp(-relu(-v2)) - 1
                    r1 = epp.tile([P, IB], F32, tag="r1", name="r1")
                    nc.scalar.activation(r1[:], v2[:], AF.Relu, scale=-1.0)
                    r3 = epp.tile([P, IB], F32, tag="r3", name="r3")
                    nc.vector.tensor_scalar(r3[:], v2[:], 0.0, None, AluOpType.max)
                    r2 = epp.tile([P, IB], F32, tag="r2", name="r2")
                    nc.scalar.activation(r2[:], r1[:], AF.Exp, scale=-1.0)
                    nc.vector.scalar_tensor_tensor(
                        OUT[:, isl], in0=r2[:], scalar=-1.0, in1=r3[:],
                        op0=AluOpType.add, op1=AluOpType.add)

        gat_head(xT, Wt[0, 0], bt[0, 0], Alt[0, 0], art[0, 0], X1T[0], True)
        gat_head(xT, Wt[0, 1], bt[0, 1], Alt[0, 1], art[0, 1], X1T[1], True)
        gat_head(X1T, Wt[1, 0], bt[1, 0], Alt[1, 0], art[1, 0], X2T[0], False)
        gat_head(X1T, Wt[1, 1], bt[1, 1], Alt[1, 1], art[1, 1], X2T[1], False)

        # ---- transpose X2T back and store ----
        for c in range(NJ):
            ob = smallp.tile([P, F], F32, tag="ob", name="ob")
            for f in range(2):
                tp = ps_prep.tile([P, IB], F32, tag="prep", name="prep")
                nc.tensor.transpose(tp[:, 0:P], X2T[f][:, c * P:(c + 1) * P], I128[:])
                if (c + f) % 2 == 0:
                    nc.vector.tensor_copy(ob[:, f * P:(f + 1) * P], tp[:, 0:P])
                else:
                    nc.scalar.activation(ob[:, f * P:(f + 1) * P], tp[:, 0:P], AF.Copy)
            nc.sync.dma_start(out=out_d[c * P:(c + 1) * P, :], in_=ob[:])

    nc.compile()
    return nc


_CACHE = {}
LAST_RESULTS = None


def kernel(**inputs):
    global LAST_RESULTS
    from concourse.bass_utils import run_bass_kernel_spmd

    x = np.ascontiguousarray(np.asarray(inputs["x"], dtype=np.float32))
    B = x.shape[0]
    assert B == N_CORES and x.shape[1] == N and x.shape[2] == F

    if "nc" not in _CACHE:
        _CACHE["nc"] = build_nc()
    nc = _CACHE["nc"]

    base = {}
    for l in (0, 1):
        for h in (0, 1):
            base[f"W_{l}_{h}"] = np.ascontiguousarray(
                np.asarray(inputs[f"W_{l}_{h}"], dtype=np.float32))
            base[f"b_{l}_{h}"] = np.ascontiguousarray(
                np.asarray(inputs[f"b_{l}_{h}"], dtype=np.float32))
            base[f"a_{l}_{h}"] = np.ascontiguousarray(
                np.asarray(inputs[f"a_{l}_{h}"], dtype=np.float32))

    in_maps = [dict(base, x=np.ascontiguousarray(x[i])) for i in range(B)]
    res = run_bass_kernel_spmd(nc, in_maps, list(range(N_CORES)),
                               trace=bool(os.environ.get("BASS_TRACE")))
    LAST_RESULTS = res
    out = np.stack([res.results[i]["out"] for i in range(B)], axis=0)
    return out.astype(np.float32)
